# revision 44
# baseline (speedup 1.0000x reference)
"""BitLinear158 Trainium2 kernel (per-core body + host driver).

Per core: host-prepacked per-block activations xb [P, KC, bt] bf16 ->
per-token int8 quant -> fp8e4 DoubleRow matmul against host-packed ternary
weight pairs -> rescale -> y [M_LOC, N] bf16.

Matmul precision scheme (hybrid, fully deterministic):
  A DoubleRow fp8 MM ([128,2,512] rhs) streams at the moving-port limit of
  256 B/cycle -> 216 ns, the same rate as a bf16 [128,512] MM, i.e. 2 fp8
  contraction slots per cycle (157 TF/s fp8 peak). Of the 16 k-chunks,
  ND are carried as DIRECT e4m3 roundings of the int8 activations (1 slot
  each, two chunks per MM) and the rest EXACTLY as (h, l) pairs with
  xq = 16*h + l (h = e4m3(xq/16), l = xq - 16*h, |l| <= 8, both exact in
  e4m3) against weight pairs (16w, w). Even m-tiles use ND=[8,8,8,10] per
  n-tile (47 MMs), odd m-tiles [8,8,10,10] (46 MMs): 46.5 MMs per m-tile
  vs 64 bf16-equivalent. The e4m3 rounding of the direct chunks gives
  1.979e-2 relative error (verified bit-accurately in numpy against the
  reference; gate is 2e-2).

Quant pipeline (all in [k-part, token] layout; per block of <=512 tokens):
  xb    [128,16,bt] <- ONE contiguous DMA (host-prepacked [P, KC, bt])
  amaxT = abs_max tree over kc (4 DVE ops, bf16 domain)
  amaxR = gpsimd partition_all_reduce(absmax) (replicated on 128 parts)
  sT    = bf16(127 * recip_approx(amaxR));  rT = bf16(amaxR / 127)
  t     = bf16(xb * sT)  (f32 product, single bf16 round = jax semantics)
  xq8   = int8(t)        (RNE + saturate)
  xall  [128,13,2,bt] f8e4: direct chunks by one i8->f8 copy; exact chunks
        in TWO fused ops: h = f8(xq*0.0625) (any e4m3 rounding keeps
        l = xq - 16*h integer in [-8,8], so the pair stays exact),
        l = f8(h*-16 + xq) via scalar_tensor_tensor
  r_nat <- [128,128] SBUF->SBUF DMA transposes of rT + ACT f32 cast on the
        scalar queue (never blocks the DVE FIFO)
matmul:  PSUM [128m,512n] f32 += DR(xall[:,pc,:,mi], wt[:,pc,:,nt]) over
         the per-n-tile pc list.
rescale: y_sb = ACT activation(Copy, scale=r_nat) -> bf16; y DMA per
         n-tile slice on the sync queue.
wt is loaded in pc-major full-row chunks; the first four ride the scalar
queue so x0/x1 own the sync ring during the prologue, the rest alternate.
"""

import sys

sys.path.insert(0, "/opt/trn_rl_repo")

from contextlib import ExitStack

import numpy as np
import ml_dtypes

import concourse.bass as bass
import concourse.tile as tile
from concourse import bacc, mybir, bass_isa
from concourse import bass_utils

P = 128
M_LOC = 4096      # tokens per core
K = 2048          # in features
N = 2048          # out features
KC = K // P       # 16 k-chunks
# per-n-tile direct-chunk counts (ND) and the shared xall/wt pc layout:
#   pc 0..4  : direct pairs (chunks 2pc, 2pc+1)   [chunks 0..9]
#   pc 5..12 : exact pairs (16*w_c, w_c), c = pc+3 [chunks 8..15]
# Even m-tiles use ND=[8,8,8,10] (47 MMs), odd use [8,8,10,10] (46 MMs):
# 46.5 MMs avg, rel err 1.979e-2 (bit-accurate numpy sim; gate 2e-2).
ND_NT_A = [8, 8, 8, 10]
ND_NT_B = [8, 8, 10, 10]
NDMAX = 10                    # -> 5 direct pair groups
PCW = NDMAX // 2 + 8          # 13 pc groups in xall/wt
# pc list per n-tile: direct pcs 0..nd/2-1, then the exact pc for chunk c
# (c in nd..15) at index NDMAX//2 + (c - 8)
def _pcs(nds):
    return [list(range(nd // 2)) + [NDMAX // 2 + (c - 8) for c in range(nd, KC)]
            for nd in nds]
PCS_A = _pcs(ND_NT_A)
PCS_B = _pcs(ND_NT_B)
BT = 512          # max tokens per block (buffer sizing)
BTS = [128, 256, 384, 512, 512, 512, 512, 512, 512, 256]
assert sum(BTS) == M_LOC and all(b % P == 0 for b in BTS)
BSTART = [sum(BTS[:i]) for i in range(len(BTS))]
MB = len(BTS)
NT = M_LOC // P   # 32 m-tiles per core
N_TILE = 512
NTN = N // N_TILE # 4
N_CORES = 8
WARMUP_MM = 32

BF16 = mybir.dt.bfloat16
F32 = mybir.dt.float32
I8 = mybir.dt.int8
I16 = mybir.dt.int16
F8E4 = mybir.dt.float8e4


def build_kernel():
    nc = bacc.Bacc("TRN2", target_bir_lowering=False, debug=False, num_devices=N_CORES)
    xb_d = [
        nc.dram_tensor(f"x{b}", [P, KC, BTS[b]], BF16, kind="ExternalInput").ap()
        for b in range(MB)
    ]
    wT = nc.dram_tensor("wT", [PCW * P, 2, N], F8E4, kind="ExternalInput").ap()
    y = nc.dram_tensor("y", [M_LOC, N], BF16, kind="ExternalOutput").ap()

    y_tiled = y.rearrange("(t p) n -> t p n", p=P)
    wT_tiled = wT.rearrange("(c p) two n -> p c two n", p=P)

    with tile.TileContext(nc) as tc, ExitStack() as ctx:
        wbuf = ctx.enter_context(tc.tile_pool(name="wbuf", bufs=1))
        xtp = ctx.enter_context(tc.tile_pool(name="xtp", bufs=2))
        qtp = ctx.enter_context(tc.tile_pool(name="qtp", bufs=3))
        ttp = ctx.enter_context(tc.tile_pool(name="ttp", bufs=1))
        tree = ctx.enter_context(tc.tile_pool(name="tree", bufs=1))
        stat = ctx.enter_context(tc.tile_pool(name="stat", bufs=2))
        snat = ctx.enter_context(tc.tile_pool(name="snat", bufs=8))
        yout = ctx.enter_context(tc.tile_pool(name="yout", bufs=16))
        psum = ctx.enter_context(tc.tile_pool(name="psum", bufs=8, space="PSUM"))

        # block 0/1 activations issue FIRST; x0 rides the sync ring ALONE
        # (the early weight chunks go on scalar) so the latency-critical
        # cold transfer is never queued behind anything.
        xT_pre = {}
        xt0 = xtp.tile([P, KC, BT], BF16, tag="xT", name="xT")[:, :, :BTS[0]]
        # x0's kc-halves ride BOTH rings in parallel (the DMA subsystem ramps
        # over ~10us of wall time, so halving the serial bytes on each ring
        # is the only way to get block 0's data sooner); the amax tree
        # processes each half as it lands
        nc.sync.dma_start(xt0[:, : KC // 2, :], xb_d[0][:, : KC // 2, :])
        nc.scalar.dma_start(xt0[:, KC // 2 :, :], xb_d[0][:, KC // 2 :, :])
        xT_pre[0] = xt0
        # x1 splits the same way so a slow early-DMA draw can't make its
        # arrival (rather than the DVE) gate block-1's quant
        xt1 = xtp.tile([P, KC, BT], BF16, tag="xT", name="xT")[:, :, :BTS[1]]
        nc.sync.dma_start(xt1[:, : KC // 2, :], xb_d[1][:, : KC // 2, :])
        nc.scalar.dma_start(xt1[:, KC // 2 :, :], xb_d[1][:, KC // 2 :, :])
        xT_pre[1] = xt1
        # weight pairs: [128, pc, 2, n]; PCW chunk DMAs (4KB row segments) in
        # consumption order; the first four (needed at MM start) go on the
        # scalar ring so x0/x1 own the sync ring, the rest alternate.
        wt = wbuf.tile([P, PCW, 2, N], F8E4)
        for pc in range(PCW):
            eng = nc.scalar if (pc < 4 or pc % 2 == 0) else nc.sync
            eng.dma_start(wt[:, pc, :, :], wT_tiled[:, pc, :, :])

        def quant_block(b):
            bt = BTS[b]
            if b in xT_pre:
                xT = xT_pre.pop(b)
            else:
                xT = xtp.tile([P, KC, BT], BF16, tag="xT", name="xT")[:, :, :bt]
                nc.sync.dma_start(xT, xb_d[b])
            # |x| by clearing the bf16 sign bit on an int16 view; for
            # non-negative IEEE values int16 order matches value order,
            # so the max tree runs in the int16 domain. The scratch tile is
            # reused (bitcast) later in the block as the bf16 product `t` —
            # the abs values are dead by then.
            scr = ttp.tile([P, KC, BT], I16, tag="scr", name="scr")[:, :, :bt]
            tr1 = tree.tile([P, 8, BT], I16, tag="tr1", name="tr1")[:, :, :bt]
            nc.vector.tensor_scalar(
                scr, xT.bitcast(I16), 0x7FFF, None,
                op0=mybir.AluOpType.bitwise_and,
            )
            nc.vector.tensor_tensor(
                tr1, scr[:, 0::2, :], scr[:, 1::2, :], mybir.AluOpType.max
            )
            tr2 = tree.tile([P, 4, BT], I16, tag="tr2", name="tr2")[:, :, :bt]
            nc.vector.tensor_tensor(
                tr2, tr1[:, 0::2, :], tr1[:, 1::2, :], mybir.AluOpType.max
            )
            tr3 = tree.tile([P, 2, BT], I16, tag="tr3", name="tr3")[:, :, :bt]
            nc.vector.tensor_tensor(
                tr3, tr2[:, 0::2, :], tr2[:, 1::2, :], mybir.AluOpType.max
            )
            # last tree level in the bf16 float domain (values are positive,
            # identical order) with f32 output — folds the upcast for free
            amax_f = stat.tile([P, BT], F32, tag="amax_f", name="amax_f")[:, :bt]
            nc.vector.tensor_tensor(
                amax_f, tr3[:, 0, :].bitcast(BF16), tr3[:, 1, :].bitcast(BF16),
                mybir.AluOpType.max,
            )
            # replicate the per-token max across all 128 partitions
            amax_r = stat.tile([P, BT], F32, tag="amax_r", name="amax_r")[:, :bt]
            nc.gpsimd.partition_all_reduce(
                amax_r, amax_f, channels=P, reduce_op=bass_isa.ReduceOp.absmax
            )
            q = stat.tile([P, BT], F32, tag="q", name="q")[:, :bt]
            nc.vector.reciprocal_approx_fast(q, amax_r)
            sT = stat.tile([P, BT], BF16, tag="sT", name="sT")[:, :bt]
            nc.vector.tensor_scalar_mul(sT, q, 127.0)
            # transposed-layout output rescale factors r = 1/s ~= amax/127
            rT = stat.tile([P, BT], BF16, tag="rT", name="rT")[:, :bt]
            nc.vector.tensor_scalar_mul(rT, amax_r, 1.0 / 127.0)

            # quantize: bf16 product -> int8 (RNE+sat) -> fp8 chunk slots
            t = scr.bitcast(BF16)
            nc.vector.tensor_tensor(
                t, xT, sT[:, None, :].to_broadcast([P, KC, bt]),
                mybir.AluOpType.mult,
            )
            xq8 = ttp.tile([P, KC, BT], I8, tag="xq8", name="xq8")[:, :, :bt]
            nc.vector.tensor_copy(xq8, t)

            xall = qtp.tile([P, PCW, 2, BT], F8E4, tag="xall",
                            name="xall")[:, :, :, :bt]
            # direct chunks 0..NDMAX-1: e4m3(xq), two chunks per pair-slot
            nc.vector.tensor_copy(
                xall[:, : NDMAX // 2, :, :], xq8[:, :NDMAX, :]
            )
            # exact chunks 8..15: h = e4m3(t/16), computed from t (not xq8)
            # so h and xq8 derive in parallel from t. l = xq - 16h stays a
            # small dyadic in [-8,8], always exact in e4m3, so the pair
            # (h, l) against (16w, w) is exact regardless of h's rounding.
            nc.vector.tensor_scalar_mul(
                xall[:, NDMAX // 2 :, 0, :], t[:, 8:, :], 0.0625
            )
            nc.vector.scalar_tensor_tensor(
                xall[:, NDMAX // 2 :, 1, :], xall[:, NDMAX // 2 :, 0, :],
                -16.0, xq8[:, 8:, :],
                op0=mybir.AluOpType.mult, op1=mybir.AluOpType.add,
            )

            # per-m-tile natural-layout rescale factors; transposes ride the
            # sync queue (the scalar queue is the ACT-rescale bottleneck)
            r_nat = []
            for c in range(bt // P):
                st = snat.tile([P, P], BF16, tag="st", name="st")
                nc.sync.dma_start_transpose(st[:], rT[:, c * P : (c + 1) * P])
                r32 = snat.tile([P, 1], F32, tag="r32", name="r32")
                nc.scalar.activation(
                    r32[:], st[:, 0:1], mybir.ActivationFunctionType.Copy
                )
                r_nat.append(r32)
            return xall, r_nat

        def mm_block(b, xall, r_nat):
            for mi in range(BTS[b] // P):
                mt = BSTART[b] // P + mi
                # odd tiles get the 47-MM config so the prologue m-tile (0)
                # runs the cheaper 46-MM one; the error mix stays 16/16
                pcs_nt = PCS_B if mt % 2 == 0 else PCS_A
                if b == 0:
                    # prologue m-tile: issue every nt-group's DIRECT MMs first
                    # (they only need the direct cast) so the PE has a runway
                    # while the exact-path h/l ops finish
                    pss = [psum.tile([P, N_TILE], F32, tag="ps", name="ps")
                           for _ in range(NTN)]
                    for nt in range(NTN):
                        for j, pc in enumerate(p for p in pcs_nt[nt] if p < 5):
                            nc.tensor.matmul(
                                pss[nt][:],
                                xall[:, pc, :, mi * P : (mi + 1) * P],
                                wt[:, pc, :, nt * N_TILE : (nt + 1) * N_TILE],
                                start=(j == 0), stop=False,
                                perf_mode=mybir.MatmulPerfMode.DoubleRow,
                            )
                    for nt in range(NTN):
                        expcs = [p for p in pcs_nt[nt] if p >= 5]
                        for j, pc in enumerate(expcs):
                            nc.tensor.matmul(
                                pss[nt][:],
                                xall[:, pc, :, mi * P : (mi + 1) * P],
                                wt[:, pc, :, nt * N_TILE : (nt + 1) * N_TILE],
                                start=False, stop=(j == len(expcs) - 1),
                                perf_mode=mybir.MatmulPerfMode.DoubleRow,
                            )
                        ns = slice(nt * N_TILE, (nt + 1) * N_TILE)
                        ysb = yout.tile([P, N_TILE], BF16, tag="y_sb",
                                        name="y_sb")
                        nc.scalar.activation(
                            ysb[:], pss[nt][:],
                            mybir.ActivationFunctionType.Copy,
                            scale=r_nat[mi][:],
                        )
                        nc.sync.dma_start(y_tiled[mt][:, ns], ysb[:])
                    continue
                for nt in range(NTN):
                    ps = psum.tile([P, N_TILE], F32, tag="ps", name="ps")
                    pcs = pcs_nt[nt]
                    for j, pc in enumerate(pcs):
                        nc.tensor.matmul(
                            ps[:],
                            xall[:, pc, :, mi * P : (mi + 1) * P],
                            wt[:, pc, :, nt * N_TILE : (nt + 1) * N_TILE],
                            start=(j == 0),
                            stop=(j == len(pcs) - 1),
                            perf_mode=mybir.MatmulPerfMode.DoubleRow,
                        )
                    ns = slice(nt * N_TILE, (nt + 1) * N_TILE)
                    ysb = yout.tile([P, N_TILE], BF16, tag="y_sb",
                                    name="y_sb")
                    nc.scalar.activation(
                        ysb[:], ps[:],
                        mybir.ActivationFunctionType.Copy,
                        scale=r_nat[mi][:],
                    )
                    nc.sync.dma_start(y_tiled[mt][:, ns], ysb[:])

        xq_map = {0: quant_block(0)}
        for b in range(MB):
            if b + 1 < MB:
                xq_map[b + 1] = quant_block(b + 1)
            mm_block(b, *xq_map.pop(b))

    nc.compile()
    return nc


def unpack_wpair(packed_weight: np.ndarray, weight_scale: np.ndarray) -> np.ndarray:
    planes = [((packed_weight >> (2 * i)) & 3) for i in range(4)]
    w = np.concatenate(planes, 0).astype(np.float32) - 1.0  # [N, K]
    ws = np.float32(weight_scale.reshape(-1)[0])
    wk = np.ascontiguousarray((w / ws).T)  # [K, N] f32
    wc = wk.reshape(KC, P, N)
    # direct pair-chunks pc 0..4: (chunk 2pc, chunk 2pc+1)
    direct = np.stack([wc[0:NDMAX:2], wc[1:NDMAX:2]], axis=2)  # [5, P, 2, N]
    # exact pair-chunks pc 5..12: (16*w_c, w_c) for c = 8..15
    ex = np.stack([16.0 * wc[8:], wc[8:]], axis=2)             # [8, P, 2, N]
    wall = np.concatenate([direct, ex], axis=0).reshape(PCW * P, 2, N)
    return np.ascontiguousarray(wall).astype(ml_dtypes.float8_e4m3)


_CACHE = {}


def run(x: np.ndarray, packed_weight: np.ndarray, weight_scale: np.ndarray,
        trace: bool = False, tmpdir=None):
    """x: [B, S, K] bf16 -> y [B, S, N] bf16 (full, unsharded)."""
    if "nc" not in _CACHE:
        _CACHE["nc"] = build_kernel()
    nc = _CACHE["nc"]

    B, S, D = x.shape
    M = B * S
    assert M == M_LOC * N_CORES and D == K
    wT = unpack_wpair(packed_weight, weight_scale)
    shards = np.asarray(x).reshape(N_CORES, M_LOC, K)
    in_maps = []
    for i in range(N_CORES):
        im = {"wT": wT}
        for b in range(MB):
            blk = shards[i][BSTART[b] : BSTART[b] + BTS[b]]      # [bt, K]
            im[f"x{b}"] = np.ascontiguousarray(
                blk.T.reshape(KC, P, BTS[b]).transpose(1, 0, 2)
            )
        in_maps.append(im)
    res = bass_utils.run_bass_kernel_spmd(
        nc, in_maps, core_ids=list(range(N_CORES)), trace=trace, tmpdir=tmpdir
    )
    y = np.stack([res.results[i]["y"] for i in range(N_CORES)], axis=0)
    return y.reshape(B, S, N), res


def kernel(x, packed_weight, weight_scale):
    """Harness entrypoint: FULL inputs -> FULL output.

    x: [4, 8192, 2048] bf16; packed_weight: [512, 2048] uint8;
    weight_scale: [1] bf16.  Returns [4, 8192, 2048] bf16.
    Sharding: data-parallel over tokens across the 8 NeuronCores;
    the (host-packed) ternary weight pairs are replicated.
    """
    x = np.asarray(x)
    packed_weight = np.asarray(packed_weight)
    weight_scale = np.asarray(weight_scale)
    y, _ = run(x, packed_weight, weight_scale)
    return y
